# revision 5
# baseline (speedup 1.0000x reference)
"""Trainium2 Bass kernel for nn_Attention (LayerNorm -> MHA -> out-proj).

Full (unsharded) inputs in, full output out. Internally shards across 8
NeuronCores as (batch b in 0..3) x (head-group g in 0..1): core c = 2*b + g
computes batch b, heads [g*8, g*8+8) of 16, producing a partial output
projection [2048, 1024]; the host sums the two group partials per batch and
adds b_out.

Device program (identical SPMD on all cores, all matmuls float32r):
  1. LayerNorm over x[b] in [token, dim] layout; PE-transpose xn -> xnT
     [dim, token] (gamma/beta optionally applied via host-broadcast tiles).
  2. QKV: Q^T/K^T [512, 2048] (head-major rows, 2 heads per 128-row tile)
     and V [token, 8*65] blocks (65th column per head = 1.0 so the P@V
     matmul also produces the softmax denominator row).
  3. Per head: S^T[key, q] = K_h^T.T @ Q_h^T (K=64), exp on ACT (softmax
     without max subtraction -- scores are O(6) for this distribution),
     PV: out^T[dh(+denom), q] accumulated over 16 key blocks.
     Normalize: denom -> reciprocal -> PE broadcast to 64 rows -> multiply.
  4. Projection: out_partial[token, dim] = outT.T @ w_out_g^T, DMA out.
"""

import sys

if "/opt/trn_rl_repo" not in sys.path:
    sys.path.insert(0, "/opt/trn_rl_repo")

from contextlib import ExitStack

import numpy as np

import concourse.tile as tile
from concourse import bacc, mybir
from concourse.bass_utils import run_bass_kernel_spmd
from concourse.masks import make_identity

P = 128
N_TOK = 2048
DIM = 1024
HEADS_TOTAL = 16
H = 8  # heads per core
DH = 64
GI = H * DH  # 512, per-core inner size
INNER = HEADS_TOTAL * DH  # 1024
N_CORES = 8
SCALE = DH ** -0.5
EPS = 1e-5

AF = mybir.ActivationFunctionType
AX = mybir.AxisListType
f32 = mybir.dt.float32
f32r = mybir.dt.float32r

_CACHE = {}


def build_nc(apply_gb=False):
    nc = bacc.Bacc("TRN2", target_bir_lowering=False, debug=False)
    x_d = nc.dram_tensor("x", [N_TOK, DIM], f32, kind="ExternalInput").ap()
    wq_d = nc.dram_tensor("wq", [P, 8 * GI], f32r, kind="ExternalInput").ap()
    wk_d = nc.dram_tensor("wk", [P, 8 * GI], f32r, kind="ExternalInput").ap()
    wv_d = nc.dram_tensor("wv", [P, 8 * GI], f32r, kind="ExternalInput").ap()
    wo_d = nc.dram_tensor("wo", [4, P, DIM], f32r, kind="ExternalInput").ap()
    gb_d = None
    if apply_gb:
        gb_d = (nc.dram_tensor("gbc", [P, DIM], f32, kind="ExternalInput").ap(),
                nc.dram_tensor("bbc", [P, DIM], f32, kind="ExternalInput").ap())
    out_d = nc.dram_tensor("out", [N_TOK, DIM], f32, kind="ExternalOutput").ap()

    with tile.TileContext(nc) as tc:
        _body(nc, tc, x_d, wq_d, wk_d, wv_d, wo_d, gb_d, out_d)
    nc.compile()
    return nc


def _body(nc, tc, x_d, wq_d, wk_d, wv_d, wo_d, gb_d, out_d):
    apply_gb = gb_d is not None
    # ---- raw (whole-kernel) SBUF tensors: 32B padding, no pool quantum ----
    ident = nc.alloc_sbuf_tensor("ident", [P, P], f32)
    make_identity(nc, ident[:, :])
    ones8f = nc.alloc_sbuf_tensor("ones8f", [P, H, 1], f32)
    nc.vector.memset(ones8f[:, :, :], 1.0)
    ones8r = nc.alloc_sbuf_tensor("ones8r", [P, H, 1], f32r)
    nc.vector.tensor_copy(ones8r[:, :, :], ones8f[:, :, :])
    onesrf = nc.alloc_sbuf_tensor("onesrf", [1, DH], f32)
    nc.vector.memset(onesrf[:, :], 1.0)
    onesr = nc.alloc_sbuf_tensor("onesr", [1, DH], f32r)
    nc.vector.tensor_copy(onesr[:, :], onesrf[:, :])
    epsb = nc.alloc_sbuf_tensor("epsb", [P, 1], f32)
    nc.vector.memset(epsb[:, :], EPS)
    stats = [nc.alloc_sbuf_tensor(f"stats{i}", [P, 8], f32) for i in range(2)]

    QT = [nc.alloc_sbuf_tensor(f"qtt{p}", [P, N_TOK], f32r) for p in range(4)]
    KT = [nc.alloc_sbuf_tensor(f"ktt{p}", [P, N_TOK], f32r) for p in range(4)]
    V = nc.alloc_sbuf_tensor("vt", [P, 16, H, DH + 1], f32r)
    for t in range(16):
        nc.vector.tensor_copy(V[:, t, :, DH : DH + 1], ones8r[:, :, :])

    # ---- phase A: LayerNorm + transpose + QKV projections ----
    with tc.tile_pool(name="phW", bufs=1) as phW, \
         tc.tile_pool(name="phA", bufs=1) as phA, \
         tc.tile_pool(name="phAx", bufs=2) as phAx, \
         tc.tile_pool(name="tpsum", bufs=2, space="PSUM") as tpsum, \
         tc.tile_pool(name="sqpsum", bufs=1, space="PSUM") as sqpsum, \
         tc.tile_pool(name="qpsum", bufs=4, space="PSUM") as qpsum:
        wq_sb = phW.tile([P, 8 * GI], f32r, tag="wq")
        nc.sync.dma_start(wq_sb[:], wq_d[:])
        wk_sb = phW.tile([P, 8 * GI], f32r, tag="wk")
        nc.sync.dma_start(wk_sb[:], wk_d[:])
        wv_sb = phW.tile([P, 8 * GI], f32r, tag="wv")
        nc.sync.dma_start(wv_sb[:], wv_d[:])
        if apply_gb:
            gbc = phW.tile([P, DIM], f32, tag="gbc")
            nc.sync.dma_start(gbc[:], gb_d[0][:])
            bbc = phW.tile([P, DIM], f32, tag="bbc")
            nc.sync.dma_start(bbc[:], gb_d[1][:])

        n_stage = 8 if apply_gb else 4  # token stages
        stok = N_TOK // n_stage
        tpst = stok // P  # token tiles per stage
        for q in range(n_stage):
            xnT = phA.tile([P, 8, stok], f32r, tag="xnt", name="xnt")
            for tt in range(tpst):
                t = q * tpst + tt
                st = stats[t % 2]
                s, nmu = st[:, 0:1], st[:, 1:2]
                ssq, std, rstd = st[:, 2:3], st[:, 3:4], st[:, 4:5]
                xt = phAx.tile([P, DIM], f32, tag="x", name="x")
                nc.sync.dma_start(xt[:], x_d[t * P : (t + 1) * P, :])
                nc.vector.reduce_sum(s, xt[:], axis=AX.X)
                nc.scalar.mul(nmu, s, -1.0 / DIM)
                nc.vector.tensor_scalar_add(xt[:], xt[:], nmu)
                sq = sqpsum.tile([P, DIM], f32, tag="sq", name="sq")
                nc.scalar.activation(sq[:], xt[:], AF.Square, accum_out=ssq)
                nc.scalar.activation(std, ssq, AF.Sqrt, scale=1.0 / DIM,
                                     bias=epsb[:, :])
                nc.vector.reciprocal(rstd, std)
                nc.scalar.activation(xt[:], xt[:], AF.Copy, scale=rstd)
                if apply_gb:
                    nc.vector.tensor_mul(xt[:], xt[:], gbc[:])
                    nc.vector.tensor_add(xt[:], xt[:], bbc[:])
                for d in range(8):
                    tp = tpsum.tile([P, P], f32, tag="tp", name="tp")
                    nc.tensor.transpose(tp[:], xt[:, d * P : (d + 1) * P],
                                        ident[:, :])
                    nc.vector.tensor_copy(xnT[:, d, tt * P : (tt + 1) * P], tp[:])
            # Q^T / K^T pieces: [128 rows of head-features, stok tokens]
            for p in range(4):
                for wsb, dstT in ((wq_sb, QT), (wk_sb, KT)):
                    ps = qpsum.tile([P, 512], f32, tag="qp", name="qp")
                    for d in range(8):
                        lo = d * GI + p * P
                        nc.tensor.matmul(ps[:, 0:stok], wsb[:, lo : lo + P],
                                         xnT[:, d, :],
                                         start=(d == 0), stop=(d == 7))
                    nc.scalar.copy(dstT[p][:, q * stok : (q + 1) * stok],
                                   ps[:, 0:stok])
            # V pieces: [128 tokens, 512 features]
            for tt in range(tpst):
                t = q * tpst + tt
                ps = qpsum.tile([P, 512], f32, tag="qp", name="qp")
                for d in range(8):
                    nc.tensor.matmul(ps[:], xnT[:, d, tt * P : (tt + 1) * P],
                                     wv_sb[:, d * GI : (d + 1) * GI],
                                     start=(d == 0), stop=(d == 7))
                nc.vector.tensor_copy(
                    V[:, t, :, 0:DH],
                    ps[:].rearrange("p (h w) -> p h w", w=DH))

    # ---- attention ----
    outT = [nc.alloc_sbuf_tensor(f"ott{p}", [P, N_TOK], f32r) for p in range(4)]
    with tc.tile_pool(name="attS", bufs=4) as attS, \
         tc.tile_pool(name="attN", bufs=1) as attN, \
         tc.tile_pool(name="attB", bufs=1) as attB, \
         tc.tile_pool(name="spsum", bufs=2, space="PSUM") as spsum, \
         tc.tile_pool(name="pvpsum", bufs=1, space="PSUM") as pvpsum:
        for h in range(H):
            p_, hh = h // 2, h % 2
            r0, r1 = hh * DH, (hh + 1) * DH
            pv = pvpsum.tile([P, N_TOK], f32, tag="pv", name="pv")
            for kb in range(16):
                for qh in range(2):
                    sps = spsum.tile([P, 1024], f32, tag="sp", name="sp")
                    for qq in range(2):
                        qcol = qh * 1024 + qq * 512
                        nc.tensor.matmul(
                            sps[:, qq * 512 : (qq + 1) * 512],
                            KT[p_][r0:r1, kb * P : (kb + 1) * P],
                            QT[p_][r0:r1, qcol : qcol + 512],
                            start=True, stop=True)
                    es = attS.tile([P, 1024], f32r, tag="es", name="es")
                    nc.scalar.activation(es[:], sps[:], AF.Exp, scale=SCALE)
                    for qq in range(2):
                        qcol = qh * 1024 + qq * 512
                        nc.tensor.matmul(
                            pv[0 : DH + 1, qcol : qcol + 512],
                            V[:, kb, h, :],
                            es[:, qq * 512 : (qq + 1) * 512],
                            start=(kb == 0), stop=(kb == 15))
            # normalize: denom (row 64) -> recip -> PE broadcast -> multiply
            den = attN.tile([1, N_TOK], f32, tag="den", name="den")
            nc.scalar.copy(den[:], pv[DH : DH + 1, :])
            recr = attN.tile([1, N_TOK], f32r, tag="recr", name="recr")
            with nc.allow_low_precision(reason="f32r denom for PE broadcast"):
                nc.vector.reciprocal(recr[:], den[:])
            for bh in range(2):
                bc = spsum.tile([P, 1024], f32, tag="sp", name="bc")
                for qq in range(2):
                    col = bh * 1024 + qq * 512
                    nc.tensor.matmul(bc[0:DH, qq * 512 : (qq + 1) * 512],
                                     onesr[:, :], recr[:, col : col + 512],
                                     start=True, stop=True)
                bcs = attB.tile([DH, 1024], f32, tag="bcs", name="bcs")
                nc.scalar.copy(bcs[:], bc[0:DH, :])
                nc.vector.tensor_mul(
                    outT[p_][r0:r1, bh * 1024 : (bh + 1) * 1024],
                    pv[0:DH, bh * 1024 : (bh + 1) * 1024],
                    bcs[:])

    # ---- output projection ----
    with tc.tile_pool(name="wop", bufs=1) as wop, \
         tc.tile_pool(name="proj", bufs=2) as proj, \
         tc.tile_pool(name="ppsum", bufs=2, space="PSUM") as ppsum:
        wo_sb = [wop.tile([P, DIM], f32r, tag=f"wo{p}", name=f"wo{p}")
                 for p in range(4)]
        for p in range(4):
            nc.sync.dma_start(wo_sb[p][:], wo_d[p])
        for t in range(16):
            pp = ppsum.tile([P, DIM], f32, tag="pp", name="pp")
            for p in range(4):
                for nn in range(2):
                    nc.tensor.matmul(pp[:, nn * 512 : (nn + 1) * 512],
                                     outT[p][:, t * P : (t + 1) * P],
                                     wo_sb[p][:, nn * 512 : (nn + 1) * 512],
                                     start=(p == 0), stop=(p == 3))
            ob = proj.tile([P, DIM], f32, tag="ob", name="ob")
            nc.scalar.copy(ob[:], pp[:])
            nc.sync.dma_start(out_d[t * P : (t + 1) * P, :], ob[:])


def _host_prep(x, ln_gamma, ln_beta, w_qkv, w_out, apply_gb):
    """Build per-core input maps."""

    def wchunks(w):  # w: [GI, DIM] rows=features -> [128, 8*512] lhsT chunks
        wt = np.ascontiguousarray(w.T, dtype=np.float32)  # [DIM, GI]
        return np.concatenate([wt[d * P : (d + 1) * P, :] for d in range(8)],
                              axis=1)

    in_maps = []
    for b in range(4):
        for g in range(2):
            lo, hi = g * GI, (g + 1) * GI
            m = {
                "x": np.ascontiguousarray(x[b], dtype=np.float32),
                "wq": wchunks(w_qkv[lo:hi, :]),
                "wk": wchunks(w_qkv[INNER + lo : INNER + hi, :]),
                "wv": wchunks(w_qkv[2 * INNER + lo : 2 * INNER + hi, :]),
                "wo": np.ascontiguousarray(
                    w_out[:, lo:hi].T.reshape(4, P, DIM), dtype=np.float32),
            }
            if apply_gb:
                m["gbc"] = np.ascontiguousarray(
                    np.broadcast_to(ln_gamma[None, :], (P, DIM)),
                    dtype=np.float32)
                m["bbc"] = np.ascontiguousarray(
                    np.broadcast_to(ln_beta[None, :], (P, DIM)),
                    dtype=np.float32)
            in_maps.append(m)
    return in_maps


def _run(inputs, trace=False):
    ln_gamma = np.asarray(inputs["ln_gamma"], dtype=np.float32)
    ln_beta = np.asarray(inputs["ln_beta"], dtype=np.float32)
    apply_gb = bool((ln_gamma != 1.0).any() or (ln_beta != 0.0).any())
    key = ("nc", apply_gb)
    if key not in _CACHE:
        _CACHE[key] = build_nc(apply_gb=apply_gb)
    nc = _CACHE[key]
    in_maps = _host_prep(inputs["x"], ln_gamma, ln_beta,
                         inputs["w_qkv"], inputs["w_out"], apply_gb)
    res = run_bass_kernel_spmd(nc, in_maps, list(range(N_CORES)), trace=trace)
    b_out = np.asarray(inputs["b_out"], dtype=np.float32)
    out = np.empty((4, N_TOK, DIM), dtype=np.float32)
    for b in range(4):
        out[b] = (res.results[2 * b]["out"] + res.results[2 * b + 1]["out"]
                  + b_out[None, :])
    return out, res


def kernel(**inputs):
    out, _ = _run(inputs, trace=False)
    return out


def kernel_profiled(**inputs):
    out, res = _run(inputs, trace=True)
    return out, res


# revision 15
# speedup vs baseline: 1.3298x; 1.3298x over previous
"""Trainium2 Bass kernel for nn_Attention (LayerNorm -> MHA -> out-proj).

Full (unsharded) inputs in, full output out. Internally shards across 8
NeuronCores as (batch b in 0..3) x (head-group g in 0..1): core c = 2*b + g
computes batch b, heads [g*8, g*8+8) of 16, producing a partial output
projection [2048, 1024]; the host sums the two group partials per batch and
adds b_out.

Device program (identical SPMD on all cores, all matmuls float32r):
  1. LayerNorm over x[b] in [token, dim] layout; PE-transpose xn -> xnT
     [dim, token] (gamma/beta optionally applied via host-broadcast tiles).
  2. QKV: Q^T/K^T [512, 2048] (head-major rows, 2 heads per 128-row tile)
     and V [token, 8*65] blocks (65th column per head = 1.0 so the P@V
     matmul also produces the softmax denominator row).
  3. Per head: S^T[key, q] = K_h^T.T @ Q_h^T (K=64), exp on ACT (softmax
     without max subtraction -- scores are O(6) for this distribution),
     PV: out^T[dh(+denom), q] accumulated over 16 key blocks.
     Normalize: denom -> reciprocal -> PE broadcast to 64 rows -> multiply.
  4. Projection: out_partial[token, dim] = outT.T @ w_out_g^T, DMA out.
"""

import sys

if "/opt/trn_rl_repo" not in sys.path:
    sys.path.insert(0, "/opt/trn_rl_repo")

from contextlib import ExitStack

import numpy as np

import concourse.tile as tile
from concourse import bacc, mybir
from concourse.bass_utils import run_bass_kernel_spmd
from concourse.masks import make_identity

P = 128
N_TOK = 2048
DIM = 1024
HEADS_TOTAL = 16
H = 8  # heads per core
DH = 64
GI = H * DH  # 512, per-core inner size
INNER = HEADS_TOTAL * DH  # 1024
N_CORES = 8
SCALE = DH ** -0.5
EPS = 1e-5

AF = mybir.ActivationFunctionType
AX = mybir.AxisListType
f32 = mybir.dt.float32
f32r = mybir.dt.float32r

_CACHE = {}


def build_nc(apply_gb=False):
    nc = bacc.Bacc("TRN2", target_bir_lowering=False, debug=False)
    x_d = nc.dram_tensor("x", [N_TOK, DIM], f32, kind="ExternalInput").ap()
    wq_d = nc.dram_tensor("wq", [P, 8 * GI], f32r, kind="ExternalInput").ap()
    wk_d = nc.dram_tensor("wk", [P, 8 * GI], f32r, kind="ExternalInput").ap()
    wv_d = nc.dram_tensor("wv", [P, 8 * GI], f32r, kind="ExternalInput").ap()
    wo_d = nc.dram_tensor("wo", [4, P, DIM], f32r, kind="ExternalInput").ap()
    gb_d = None
    if apply_gb:
        gb_d = (nc.dram_tensor("gbc", [P, DIM], f32, kind="ExternalInput").ap(),
                nc.dram_tensor("bbc", [P, DIM], f32, kind="ExternalInput").ap())
    out_d = nc.dram_tensor("out", [N_TOK, DIM], f32, kind="ExternalOutput").ap()

    denb_d = nc.dram_tensor("denb", [H, N_TOK], f32).ap()
    with tile.TileContext(nc) as tc:
        _body(nc, tc, x_d, wq_d, wk_d, wv_d, wo_d, gb_d, out_d, denb_d)
    nc.compile()
    return nc


def _body(nc, tc, x_d, wq_d, wk_d, wv_d, wo_d, gb_d, out_d, denb_d):
    apply_gb = gb_d is not None
    # ---- raw (whole-kernel) SBUF tensors: 32B padding, no pool quantum ----
    ident = nc.alloc_sbuf_tensor("ident", [P, P], f32)
    make_identity(nc, ident[:, :])
    ones8f = nc.alloc_sbuf_tensor("ones8f", [P, H, 1], f32)
    nc.vector.memset(ones8f[:, :, :], 1.0)
    ones8r = nc.alloc_sbuf_tensor("ones8r", [P, H, 1], f32r)
    nc.vector.tensor_copy(ones8r[:, :, :], ones8f[:, :, :])
    onesrf = nc.alloc_sbuf_tensor("onesrf", [1, DH], f32)
    nc.vector.memset(onesrf[:, :], 1.0)
    onesr = nc.alloc_sbuf_tensor("onesr", [1, DH], f32r)
    nc.vector.tensor_copy(onesr[:, :], onesrf[:, :])
    epsb = nc.alloc_sbuf_tensor("epsb", [P, 1], f32)
    nc.vector.memset(epsb[:, :], EPS)
    # selector for denom broadcast: self8r[k, h, :] = 1.0 iff k == h
    self8f = nc.alloc_sbuf_tensor("self8f", [H, H, DH], f32)
    nc.gpsimd.memset(self8f[:, :, :], 0.0)
    nc.gpsimd.affine_select(out=self8f[:, :, :], in_=self8f[:, :, :],
                            compare_op=mybir.AluOpType.not_equal, fill=1.0,
                            base=0, pattern=[[-1, H], [0, DH]],
                            channel_multiplier=1)
    self8r = nc.alloc_sbuf_tensor("self8r", [H, H, DH], f32r)
    nc.vector.tensor_copy(self8r[:, :, :], self8f[:, :, :])
    stats = [nc.alloc_sbuf_tensor(f"stats{i}", [P, 8], f32) for i in range(2)]

    QT = [nc.alloc_sbuf_tensor(f"qtt{p}", [P, N_TOK], f32r) for p in range(4)]
    KT = [nc.alloc_sbuf_tensor(f"ktt{p}", [P, N_TOK], f32r) for p in range(4)]
    V = nc.alloc_sbuf_tensor("vt", [P, 16, H, DH + 1], f32r)
    for t in range(16):
        nc.vector.tensor_copy(V[:, t, :, DH : DH + 1], ones8r[:, :, :])

    # ---- phase A: LayerNorm + transpose + QKV projections ----
    with tc.tile_pool(name="phW", bufs=1) as phW, \
         tc.tile_pool(name="phA", bufs=1) as phA, \
         tc.tile_pool(name="phAx", bufs=2) as phAx, \
         tc.tile_pool(name="tpsum", bufs=2, space="PSUM") as tpsum, \
         tc.tile_pool(name="sqpsum", bufs=1, space="PSUM") as sqpsum, \
         tc.tile_pool(name="qpsum", bufs=4, space="PSUM") as qpsum:
        wq_sb = phW.tile([P, 8 * GI], f32r, tag="wq")
        nc.sync.dma_start(wq_sb[:], wq_d[:])
        wk_sb = phW.tile([P, 8 * GI], f32r, tag="wk")
        nc.sync.dma_start(wk_sb[:], wk_d[:])
        wv_sb = phW.tile([P, 8 * GI], f32r, tag="wv")
        nc.sync.dma_start(wv_sb[:], wv_d[:])
        if apply_gb:
            gbc = phW.tile([P, DIM], f32, tag="gbc")
            nc.sync.dma_start(gbc[:], gb_d[0][:])
            bbc = phW.tile([P, DIM], f32, tag="bbc")
            nc.sync.dma_start(bbc[:], gb_d[1][:])

        n_stage = 8 if apply_gb else 4  # token stages
        stok = N_TOK // n_stage
        tpst = stok // P  # token tiles per stage
        for q in range(n_stage):
            xnT = phA.tile([P, 8, stok], f32r, tag="xnt", name="xnt")
            for tt in range(tpst):
                t = q * tpst + tt
                st = stats[t % 2]
                s, nmu = st[:, 0:1], st[:, 1:2]
                ssq, std, rstd = st[:, 2:3], st[:, 3:4], st[:, 4:5]
                xt = phAx.tile([P, DIM], f32, tag="x", name="x")
                nc.sync.dma_start(xt[:], x_d[t * P : (t + 1) * P, :])
                nc.vector.reduce_sum(s, xt[:], axis=AX.X)
                nc.scalar.mul(nmu, s, -1.0 / DIM)
                nc.vector.tensor_scalar_add(xt[:], xt[:], nmu)
                sq = sqpsum.tile([P, DIM], f32, tag="sq", name="sq")
                nc.scalar.activation(sq[:], xt[:], AF.Square, accum_out=ssq)
                nc.scalar.activation(std, ssq, AF.Sqrt, scale=1.0 / DIM,
                                     bias=epsb[:, :])
                nc.vector.reciprocal(rstd, std)
                nc.scalar.activation(xt[:], xt[:], AF.Copy, scale=rstd)
                if apply_gb:
                    nc.vector.tensor_mul(xt[:], xt[:], gbc[:])
                    nc.vector.tensor_add(xt[:], xt[:], bbc[:])
                for d in range(8):
                    tp = tpsum.tile([P, P], f32, tag="tp", name="tp")
                    nc.tensor.transpose(tp[:], xt[:, d * P : (d + 1) * P],
                                        ident[:, :])
                    nc.vector.tensor_copy(xnT[:, d, tt * P : (tt + 1) * P], tp[:])
            # Q^T / K^T pieces: [128 rows of head-features, stok tokens]
            for p in range(4):
                for wsb, dstT in ((wq_sb, QT), (wk_sb, KT)):
                    ps = qpsum.tile([P, 512], f32, tag="qp", name="qp")
                    for d in range(8):
                        lo = d * GI + p * P
                        nc.tensor.matmul(ps[:, 0:stok], wsb[:, lo : lo + P],
                                         xnT[:, d, :],
                                         start=(d == 0), stop=(d == 7))
                    nc.scalar.copy(dstT[p][:, q * stok : (q + 1) * stok],
                                   ps[:, 0:stok])
            # V pieces: [128 tokens, 512 features]
            for tt in range(tpst):
                t = q * tpst + tt
                ps = qpsum.tile([P, 512], f32, tag="qp", name="qp")
                for d in range(8):
                    nc.tensor.matmul(ps[:], xnT[:, d, tt * P : (tt + 1) * P],
                                     wv_sb[:, d * GI : (d + 1) * GI],
                                     start=(d == 0), stop=(d == 7))
                nc.vector.tensor_copy(
                    V[:, t, :, 0:DH],
                    ps[:].rearrange("p (h w) -> p h w", w=DH))

    # ---- attention ----
    outT = [nc.alloc_sbuf_tensor(f"ott{p}", [P, N_TOK], f32r) for p in range(4)]
    with tc.tile_pool(name="attS", bufs=6) as attS, \
         tc.tile_pool(name="attN", bufs=1) as attN, \
         tc.tile_pool(name="attB", bufs=2) as attB, \
         tc.tile_pool(name="spsum", bufs=2, space="PSUM") as spsum, \
         tc.tile_pool(name="pvpsum", bufs=1, space="PSUM") as pvpsum:
        den_all = attN.tile([H, N_TOK], f32, tag="den_all", name="den_all")
        recr_all = attN.tile([H, N_TOK], f32r, tag="recr_all", name="recr_all")
        for h in range(H):
            p_, hh = h // 2, h % 2
            r0, r1 = hh * DH, (hh + 1) * DH
            pv = pvpsum.tile([P, N_TOK], f32, tag="pv", name="pv")
            for kb in range(16):
                for qh in range(2):
                    sps = spsum.tile([P, 1024], f32, tag="sp", name="sp")
                    for qq in range(2):
                        qcol = qh * 1024 + qq * 512
                        nc.tensor.matmul(
                            sps[:, qq * 512 : (qq + 1) * 512],
                            KT[p_][r0:r1, kb * P : (kb + 1) * P],
                            QT[p_][r0:r1, qcol : qcol + 512],
                            start=True, stop=True)
                    es = attS.tile([P, 1024], f32r, tag="es", name="es")
                    nc.scalar.activation(es[:], sps[:], AF.Exp, scale=SCALE)
                    for qq in range(2):
                        qcol = qh * 1024 + qq * 512
                        nc.tensor.matmul(
                            pv[0 : DH + 1, qcol : qcol + 512],
                            V[:, kb, h, :],
                            es[:, qq * 512 : (qq + 1) * 512],
                            start=(kb == 0), stop=(kb == 15))
            # evacuate PV promptly so the next head's PV can start: rows
            # 0..63 -> outT (unnormalized), row 64 (denominator) -> DRAM
            # bounce (engines can't write partition base h, DMA can)
            nc.vector.tensor_copy(outT[p_][r0:r1, :], pv[0:DH, :])
            dstage = attB.tile([1, N_TOK], f32, tag="dstage", name="dstage")
            nc.scalar.copy(dstage[:], pv[DH : DH + 1, :])
            nc.sync.dma_start(denb_d[h : h + 1, :], dstage[:])
        # batched normalization: one 8-lane reciprocal, then per-head
        # PE broadcast of 1/denom and in-place multiply on outT
        nc.sync.dma_start(den_all[:], denb_d[:])
        with nc.allow_low_precision(reason="f32r denom for PE broadcast"):
            nc.vector.reciprocal(recr_all[:, :], den_all[:, :])
        for h in range(H):
            p_, hh = h // 2, h % 2
            r0, r1 = hh * DH, (hh + 1) * DH
            for bh in range(2):
                bc = spsum.tile([P, 1024], f32, tag="sp", name="bc")
                for qq in range(2):
                    col = bh * 1024 + qq * 512
                    nc.tensor.matmul(bc[0:DH, qq * 512 : (qq + 1) * 512],
                                     self8r[:, h, :],
                                     recr_all[:, col : col + 512],
                                     start=True, stop=True)
                bcs = attB.tile([P, 1024], f32, tag="bcs", name="bcs")
                nc.scalar.copy(bcs[r0:r1, :], bc[0:DH, :])
                nc.vector.tensor_mul(
                    outT[p_][r0:r1, bh * 1024 : (bh + 1) * 1024],
                    outT[p_][r0:r1, bh * 1024 : (bh + 1) * 1024],
                    bcs[r0:r1, :])

    # ---- output projection ----
    with tc.tile_pool(name="wop", bufs=1) as wop, \
         tc.tile_pool(name="proj", bufs=2) as proj, \
         tc.tile_pool(name="ppsum", bufs=2, space="PSUM") as ppsum:
        wo_sb = [wop.tile([P, DIM], f32r, tag=f"wo{p}", name=f"wo{p}")
                 for p in range(4)]
        for p in range(4):
            nc.sync.dma_start(wo_sb[p][:], wo_d[p])
        for t in range(16):
            pp = ppsum.tile([P, DIM], f32, tag="pp", name="pp")
            for p in range(4):
                for nn in range(2):
                    nc.tensor.matmul(pp[:, nn * 512 : (nn + 1) * 512],
                                     outT[p][:, t * P : (t + 1) * P],
                                     wo_sb[p][:, nn * 512 : (nn + 1) * 512],
                                     start=(p == 0), stop=(p == 3))
            ob = proj.tile([P, DIM], f32, tag="ob", name="ob")
            nc.scalar.copy(ob[:], pp[:])
            nc.sync.dma_start(out_d[t * P : (t + 1) * P, :], ob[:])


def _host_prep(x, ln_gamma, ln_beta, w_qkv, w_out, apply_gb):
    """Build per-core input maps."""

    def wchunks(w):  # w: [GI, DIM] rows=features -> [128, 8*512] lhsT chunks
        wt = np.ascontiguousarray(w.T, dtype=np.float32)  # [DIM, GI]
        return np.concatenate([wt[d * P : (d + 1) * P, :] for d in range(8)],
                              axis=1)

    in_maps = []
    for b in range(4):
        for g in range(2):
            lo, hi = g * GI, (g + 1) * GI
            m = {
                "x": np.ascontiguousarray(x[b], dtype=np.float32),
                "wq": wchunks(w_qkv[lo:hi, :]),
                "wk": wchunks(w_qkv[INNER + lo : INNER + hi, :]),
                "wv": wchunks(w_qkv[2 * INNER + lo : 2 * INNER + hi, :]),
                "wo": np.ascontiguousarray(
                    w_out[:, lo:hi].T.reshape(4, P, DIM), dtype=np.float32),
            }
            if apply_gb:
                m["gbc"] = np.ascontiguousarray(
                    np.broadcast_to(ln_gamma[None, :], (P, DIM)),
                    dtype=np.float32)
                m["bbc"] = np.ascontiguousarray(
                    np.broadcast_to(ln_beta[None, :], (P, DIM)),
                    dtype=np.float32)
            in_maps.append(m)
    return in_maps


def _run(inputs, trace=False):
    ln_gamma = np.asarray(inputs["ln_gamma"], dtype=np.float32)
    ln_beta = np.asarray(inputs["ln_beta"], dtype=np.float32)
    apply_gb = bool((ln_gamma != 1.0).any() or (ln_beta != 0.0).any())
    key = ("nc", apply_gb)
    if key not in _CACHE:
        _CACHE[key] = build_nc(apply_gb=apply_gb)
    nc = _CACHE[key]
    in_maps = _host_prep(inputs["x"], ln_gamma, ln_beta,
                         inputs["w_qkv"], inputs["w_out"], apply_gb)
    res = run_bass_kernel_spmd(nc, in_maps, list(range(N_CORES)), trace=trace)
    b_out = np.asarray(inputs["b_out"], dtype=np.float32)
    out = np.empty((4, N_TOK, DIM), dtype=np.float32)
    for b in range(4):
        out[b] = (res.results[2 * b]["out"] + res.results[2 * b + 1]["out"]
                  + b_out[None, :])
    return out, res


def kernel(**inputs):
    out, _ = _run(inputs, trace=False)
    return out


def kernel_profiled(**inputs):
    out, res = _run(inputs, trace=True)
    return out, res
